# revision 21
# baseline (speedup 1.0000x reference)
"""TRN2 Bass kernel: 16-head MHA (B=2, S=2048, H=1024) sharded over 8 NeuronCores.

Sharding: data-parallel over batch (2) x tensor-parallel over head groups
(4 groups of 4 heads). Each core computes its 4 heads' attention for its batch
and a partial output projection; the host sums the 4 partials per batch,
transposes, and adds the output bias.

Per-core kernel (all activations transposed, bf16 on-chip, fp32 accumulation):
  qhT[d,q] = wq.T @ qT ; khT likewise ; vh[k,d] = (vT.T @ wv) with a ones
  column appended per head (rowsum trick).  Scores are computed transposed
  (s^T[k,q]), exp on ScalarE (scale=1/8 folded in), multiplicative {0,1} mask
  on VectorE, and the AV matmul accumulates x^T[d+1,q] in PSUM where row 64
  is the softmax denominator.  r = 1/rowsum via DVE reciprocal_approx_fast
  straight off the PSUM row, partition-broadcast by an SBUF->SBUF DMA with a
  stride-0 source, then multiplied into x.  Mask tiles are DMAed on the
  sync/gpsimd queues (never ScalarE's, which must stay free for exp) and
  prefetch during the k/v projections via staged tile-pool scopes.
"""

import sys

sys.path.insert(0, "/opt/trn_rl_repo")

from collections import deque
from contextlib import ExitStack

import numpy as np
import ml_dtypes

import concourse.tile as tile
from concourse import bacc, mybir

BF16 = mybir.dt.bfloat16
F32 = mybir.dt.float32
P = 128

_PROGRAM_CACHE = {}


def build_mha_program(S=2048, HID=1024, NH=4, DK=64, QB=1024, aug=False):
    """Build + compile the per-core SPMD Bass program."""
    D = NH * DK
    assert NH % 2 == 0 and DK == 64
    SH = S // P
    HT = HID // P
    HTa = HT + (1 if aug else 0)
    QBn = S // QB
    NS = min(512, QB)
    QH = QB // NS
    NQ = S // NS
    DC = D // P
    NPAIR = NH // 2
    GW = DK + 2                  # 64 data cols + rowsum-ones col + pad (4B-aligned groups)

    nc = bacc.Bacc("TRN2", target_bir_lowering=False, debug=False)

    qT_d = nc.dram_tensor("qT", [HTa * P, S], BF16, kind="ExternalInput").ap()
    kT_d = nc.dram_tensor("kT", [HTa * P, S], BF16, kind="ExternalInput").ap()
    vT_d = nc.dram_tensor("vT", [HTa * P, S], BF16, kind="ExternalInput").ap()
    maskT_d = nc.dram_tensor("maskT", [S, S], BF16, kind="ExternalInput").ap()
    wq_d = nc.dram_tensor("wq", [HTa * P, D], BF16, kind="ExternalInput").ap()
    wk_d = nc.dram_tensor("wk", [HTa * P, D], BF16, kind="ExternalInput").ap()
    wv_d = nc.dram_tensor("wv", [HTa * P, D], BF16, kind="ExternalInput").ap()
    wo_d = nc.dram_tensor("wo", [D, HID], BF16, kind="ExternalInput").ap()
    y_d = nc.dram_tensor("y", [HID, S], F32, kind="ExternalOutput").ap()

    Exp = mybir.ActivationFunctionType.Exp

    with tile.TileContext(nc) as tc:
        with ExitStack() as ctx:
            persist = ctx.enter_context(tc.tile_pool(name="persist", bufs=1))
            qh_t = [persist.tile([P, S], BF16, tag=f"qh{d}", name=f"qh{d}")
                    for d in range(DC)]
            kh_t = [persist.tile([P, S], BF16, tag=f"kh{d}", name=f"kh{d}")
                    for d in range(DC)]
            vh_t = [persist.tile([P, NH * GW], BF16, tag=f"vh{s}", name=f"vh{s}")
                    for s in range(SH)]
            xu_t = [persist.tile([P, S], BF16, tag=f"xu{p}", name=f"xu{p}")
                    for p in range(NPAIR)]
            xn_t = [persist.tile([P, S], BF16, tag=f"xn{p}", name=f"xn{p}")
                    for p in range(NPAIR)]
            wo_t = [persist.tile([P, HID], BF16, tag=f"wo{p}", name=f"wo{p}")
                    for p in range(NPAIR)]

            for pr in range(NPAIR):
                nc.sync.dma_start(wo_t[pr][:], wo_d[pr * P:(pr + 1) * P, :])

            def load_inputs(ph, which, src_d, w_d_):
                inp = ph.enter_context(
                    tc.tile_pool(name=f"inp_{which}", bufs=1))
                wp = ph.enter_context(
                    tc.tile_pool(name=f"wp_{which}", bufs=1))
                src_t = [inp.tile([P, S], BF16, tag=f"s{i}",
                                  name=f"{which}T{i}") for i in range(HTa)]
                w_t = [wp.tile([P, D], BF16, tag=f"w{i}",
                               name=f"w_{which}{i}") for i in range(HTa)]
                return src_t, w_t, src_d, w_d_

            def issue_dmas(loaded):
                src_t, w_t, src_d, w_d_ = loaded
                for i in range(HTa):
                    sl = slice(i * P, (i + 1) * P)
                    nc.sync.dma_start(w_t[i][:], w_d_[sl, :])
                    nc.sync.dma_start(src_t[i][:], src_d[sl, :])

            def project(ps1, loaded, dst):
                src_t, w_t, _, _ = loaded
                for dc in range(DC):
                    psl = [ps1.tile([P, NS], F32, tag=f"p1_{qc}",
                                    name=f"p1_{qc}") for qc in range(NQ)]
                    for i in range(HTa):
                        for qc in range(NQ):
                            nc.tensor.matmul(
                                psl[qc][:],
                                w_t[i][:, dc * P:(dc + 1) * P],
                                src_t[i][:, qc * NS:(qc + 1) * NS],
                                start=(i == 0), stop=(i == HTa - 1))
                    for qc in range(NQ):
                        nc.vector.tensor_copy(
                            dst[dc][:, qc * NS:(qc + 1) * NS], psl[qc][:])

            def project_v(ps1, loaded):
                """vh[k, d] with ones cols (rowsum trick) surviving the
                grouped copy."""
                vT_t, wv_t, _, _ = loaded
                for sc in range(SH):
                    ps_v = ps1.tile([P, D], F32, tag="p1v", name="p1v",
                                    bufs=2)
                    for i in range(HTa):
                        nc.tensor.matmul(
                            ps_v[:],
                            vT_t[i][:, sc * P:(sc + 1) * P],
                            wv_t[i][:],
                            start=(i == 0), stop=(i == HTa - 1))
                    nc.vector.memset(vh_t[sc][:], 1.0)
                    dst_v = vh_t[sc][:].rearrange(
                        "p (h c) -> p h c", c=GW)[:, :, 0:DK]
                    src_v = ps_v[:].rearrange("p (h c) -> p h c", c=DK)
                    nc.vector.tensor_copy(dst_v, src_v)

            # phase 1: q,k input DMAs issued up front in priority order on the
            # sync queue (q gates the first matmuls); their pools close before
            # the mask tiles allocate, so SBUF fits (LIFO pool order).
            with ExitStack() as ph_qk:
                lq = load_inputs(ph_qk, "q", qT_d, wq_d)
                lk = load_inputs(ph_qk, "k", kT_d, wk_d)
                issue_dmas(lq)
                issue_dmas(lk)
                ps1 = ph_qk.enter_context(
                    tc.tile_pool(name="ps1", bufs=1, space="PSUM"))
                project(ps1, lq, qh_t)
                project(ps1, lk, kh_t)

            # mask tiles allocate once q/k SBUF is free; their DMAs go on the
            # sync queue strictly AFTER the vT loads so v-proj (which gates
            # the PE) wins the bandwidth race, and ScalarE stays free for exp
            mp = ctx.enter_context(tc.tile_pool(name="mask", bufs=1))
            mask_t = [mp.tile([P, S], BF16, tag=f"m{i}", name=f"m{i}")
                      for i in range(SH)]

            # phase 1b: v projection (masks stream during it)
            with ExitStack() as ph_v:
                lv = load_inputs(ph_v, "v", vT_d, wv_d)
                issue_dmas(lv)
                for i in range(SH):
                    nc.sync.dma_start(mask_t[i][:],
                                      maskT_d[i * P:(i + 1) * P, :])
                ps1v = ph_v.enter_context(
                    tc.tile_pool(name="ps1v", bufs=1, space="PSUM"))
                project_v(ps1v, lv)

            # phase 2+3+4 fused: attention per (q-block, head); after each
            # q-block completes, its normalize + output projection are
            # emitted interleaved into the NEXT q-block's attention as PE
            # filler work (the attention pipeline is ScalarE-bound).
            with ExitStack() as ph2:
                pp = ph2.enter_context(tc.tile_pool(name="pexp", bufs=8))
                pmp = ph2.enter_context(tc.tile_pool(name="pmask", bufs=12))
                rp = ph2.enter_context(tc.tile_pool(name="rp", bufs=2))
                ysb = ph2.enter_context(tc.tile_pool(name="ysb", bufs=8))
                ones_t = persist.tile([P, DK], F32, tag="ones", name="ones")
                nc.vector.memset(ones_t[:], 1.0)
                sps = ph2.enter_context(
                    tc.tile_pool(name="sps", bufs=3, space="PSUM"))
                xps = ph2.enter_context(
                    tc.tile_pool(name="xps", bufs=1, space="PSUM"))

                filler = deque()

                def run_filler(n):
                    for _ in range(n):
                        if not filler:
                            return
                        filler.popleft()()

                def attention_head(qb, h):
                    """QK -> exp -> mask -> AV (deep-lagged) for one head.

                    The x/bc PSUM slot uses a single rotating tag so the score
                    pool can run 3 deep — the PE then almost never blocks on
                    the exp drain, keeping its stream dense (HAM stays at
                    K=8/8).  One filler unit (prev q-block's output
                    projection) runs per chunk."""
                    LAG = 6
                    qsl = slice(qb * QB, (qb + 1) * QB)
                    ht, hb = divmod(h, 2)
                    hsl = slice(64 * hb, 64 * hb + 64)
                    x_ps = xps.tile([P, QB], F32, tag="x", name=f"x{h}")
                    pending = []

                    def emit_av(kc, pm_t):
                        for qh_ in range(QH):
                            nsl = slice(qh_ * NS, (qh_ + 1) * NS)
                            nc.tensor.matmul(
                                x_ps[:DK + 1, nsl],
                                vh_t[kc][:, h * GW:h * GW + DK + 1],
                                pm_t[:, nsl],
                                start=(kc == 0), stop=(kc == SH - 1),
                                skip_group_check=True)

                    for kc in range(SH):
                        s_ps = sps.tile([P, QB], F32, tag="s", name="s")
                        for qh_ in range(QH):
                            nsl = slice(qh_ * NS, (qh_ + 1) * NS)
                            # the first matmul is emitted twice (second
                            # start=True overwrites): the extra PE streaming
                            # occupancy keeps HAM at K=8/8 — an 80%-busy PE
                            # never re-warms and runs at 1.2 GHz, which costs
                            # far more than the duplicated stream
                            for rep in range(2 if qh_ == 0 else 1):
                                nc.tensor.matmul(
                                    s_ps[:, nsl],
                                    kh_t[ht][hsl, kc * P:(kc + 1) * P],
                                    qh_t[ht][hsl, qb * QB + qh_ * NS:
                                             qb * QB + (qh_ + 1) * NS],
                                    start=True, stop=True)
                        p_t = pp.tile([P, QB], BF16, tag="p", name="p")
                        nc.scalar.activation(p_t[:], s_ps[:], Exp, scale=0.125)
                        pm_t = pmp.tile([P, QB], BF16, tag="pm", name="pm")
                        nc.vector.tensor_mul(
                            pm_t[:], p_t[:], mask_t[kc][:, qsl])
                        pending.append((kc, pm_t))
                        # spread the drain over the last chunks so the final
                        # AV flush never starves ScalarE of fresh scores
                        while len(pending) > min(LAG, SH + 1 - kc):
                            emit_av(*pending.pop(0))
                        run_filler(1)
                    for item in pending:
                        emit_av(*item)
                    return x_ps

                def normalize_head(qb, h, x_ps):
                    """r = 1/rowsum off the PSUM row, partition-broadcast by a
                    K=1 fp32 matmul (ones^T @ r) into the recycled x slot,
                    then xn = xu * r.  All on-chip (no DRAM bounce)."""
                    qsl = slice(qb * QB, (qb + 1) * QB)
                    pr, hb = divmod(h, 2)
                    rows = slice(64 * hb, 64 * hb + 64)
                    r_t = rp.tile([DK + 1, QB], F32, tag="r", name=f"r{h}")
                    # single-partition slices break the custom-DVE op
                    # (probe-verified); run it over rows 0..64 — same cost,
                    # only row 64 (the rowsum) is consumed
                    nc.vector.reciprocal_approx_fast(
                        r_t[0:DK + 1, :], x_ps[0:DK + 1, :])
                    nc.vector.tensor_copy(xu_t[pr][rows, qsl], x_ps[:DK, :])
                    bc = xps.tile([P, QB], F32, tag="x", name=f"bc{h}")
                    for qh_ in range(QH):
                        nsl = slice(qh_ * NS, (qh_ + 1) * NS)
                        nc.tensor.matmul(
                            bc[64 * hb:64 * hb + 64, nsl],
                            ones_t[DK:DK + 1, :],
                            r_t[DK:DK + 1, nsl],
                            start=True, stop=True)
                    nc.vector.tensor_mul(
                        xn_t[pr][rows, qsl], xu_t[pr][rows, qsl],
                        bc[rows, :])

                def make_oproj_unit(qb, hc, qh_):
                    def unit():
                        y_ps = sps.tile([P, NS], F32, tag="s", name="yps")
                        qc0 = qb * QH + qh_
                        for pr in range(NPAIR):
                            nc.tensor.matmul(
                                y_ps[:],
                                wo_t[pr][:, hc * P:(hc + 1) * P],
                                xn_t[pr][:, qc0 * NS:(qc0 + 1) * NS],
                                start=(pr == 0), stop=(pr == NPAIR - 1))
                        y_sb = ysb.tile([P, NS], F32, tag="ysb", name="ysb")
                        nc.vector.tensor_copy(y_sb[:], y_ps[:])
                        nc.gpsimd.dma_start(
                            y_d[hc * P:(hc + 1) * P,
                                qc0 * NS:(qc0 + 1) * NS],
                            y_sb[:])
                    return unit

                for qb in range(QBn):
                    for h in range(NH):
                        x_ps = attention_head(qb, h)
                        normalize_head(qb, h, x_ps)
                    for hc in range(HT):
                        for qh_ in range(QH):
                            filler.append(make_oproj_unit(qb, hc, qh_))
                run_filler(len(filler))

    nc.compile()
    return nc


def make_in_maps(q, k, v, mask, Wq, bq, Wk, bk, Wv, bv, Wo,
                 n_cores=8, NH=4, DK=64, aug=False):
    bf = ml_dtypes.bfloat16
    B, S, HID = q.shape
    D = NH * DK
    n_hg = n_cores // B

    def with_aug(xT, bias_row):
        pad = np.zeros((P, xT.shape[1]), xT.dtype)
        pad[0, :] = bias_row
        return np.concatenate([xT, pad], axis=0)

    per_batch = {}
    for b in range(B):
        qT = np.ascontiguousarray(q[b].T).astype(bf)
        kT = np.ascontiguousarray(k[b].T).astype(bf)
        vT = np.ascontiguousarray(v[b].T).astype(bf)
        if aug:
            one = np.ones((S,), np.float32).astype(bf)
            qT, kT, vT = with_aug(qT, one), with_aug(kT, one), with_aug(vT, one)
        per_batch[b] = (qT, kT, vT,
                        np.ascontiguousarray(mask[b, 0].T != 0).astype(bf))

    in_maps = []
    for core in range(n_cores):
        b, hg = divmod(core, n_hg)
        hsl = slice(hg * D, (hg + 1) * D)
        wq = Wq[:, hsl].astype(bf)
        wk = Wk[:, hsl].astype(bf)
        wv = Wv[:, hsl].astype(bf)
        if aug:
            wq = with_aug(wq, bq[hsl].astype(bf))
            wk = with_aug(wk, bk[hsl].astype(bf))
            wv = with_aug(wv, bv[hsl].astype(bf))
        qT, kT, vT, mT = per_batch[b]
        in_maps.append(dict(
            qT=qT, kT=kT, vT=vT, maskT=mT,
            wq=np.ascontiguousarray(wq), wk=np.ascontiguousarray(wk),
            wv=np.ascontiguousarray(wv),
            wo=np.ascontiguousarray(Wo[hsl, :]).astype(bf),
        ))
    return in_maps


def combine_outputs(results, B, S, HID, bo, n_cores=8):
    n_hg = n_cores // B
    out = np.zeros((B, S, HID), np.float32)
    for core in range(n_cores):
        b = core // n_hg
        out[b] += results[core]["y"].T
    return out + bo.astype(np.float32)


def run_mha(q, k, v, mask, Wq, bq, Wk, bk, Wv, bv, Wo, bo, trace=False):
    from concourse.bass_utils import run_bass_kernel_spmd

    B, S, HID = q.shape
    n_cores = 8
    aug = bool(np.any(bq) or np.any(bk) or np.any(bv))
    key = (S, HID, aug)
    if key not in _PROGRAM_CACHE:
        _PROGRAM_CACHE[key] = build_mha_program(S=S, HID=HID, aug=aug)
    nc = _PROGRAM_CACHE[key]
    in_maps = make_in_maps(q, k, v, mask, Wq, bq, Wk, bk, Wv, bv, Wo,
                           n_cores=n_cores, aug=aug)
    res = run_bass_kernel_spmd(nc, in_maps, list(range(n_cores)), trace=trace)
    out = combine_outputs(res.results, B, S, HID, bo, n_cores=n_cores)
    return out, res


def kernel(q, k, v, mask, Wq, bq, Wk, bk, Wv, bv, Wo, bo):
    q = np.asarray(q, np.float32)
    k = np.asarray(k, np.float32)
    v = np.asarray(v, np.float32)
    mask = np.asarray(mask)
    out, _ = run_mha(q, k, v, mask,
                     np.asarray(Wq, np.float32), np.asarray(bq, np.float32),
                     np.asarray(Wk, np.float32), np.asarray(bk, np.float32),
                     np.asarray(Wv, np.float32), np.asarray(bv, np.float32),
                     np.asarray(Wo, np.float32), np.asarray(bo, np.float32))
    return out


# revision 22
# speedup vs baseline: 1.1446x; 1.1446x over previous
"""TRN2 Bass kernel: 16-head MHA (B=2, S=2048, H=1024) sharded over 8 NeuronCores.

Sharding: data-parallel over batch (2) x tensor-parallel over head groups
(4 groups of 4 heads). Each core computes its 4 heads' attention for its batch
and a partial output projection; the host sums the 4 partials per batch,
transposes, and adds the output bias.

Per-core kernel (all activations transposed, bf16 on-chip, fp32 accumulation):
  qhT[d,q] = wq.T @ qT ; khT likewise ; vh[k,d] = (vT.T @ wv) with a ones
  column appended per head (rowsum trick).  Scores are computed transposed
  (s^T[k,q]), exp on ScalarE (scale=1/8 folded in), multiplicative {0,1} mask
  on VectorE, and the AV matmul accumulates x^T[d+1,q] in PSUM where row 64
  is the softmax denominator.  r = 1/rowsum via DVE reciprocal_approx_fast
  straight off the PSUM row, partition-broadcast by an SBUF->SBUF DMA with a
  stride-0 source, then multiplied into x.  Mask tiles are DMAed on the
  sync/gpsimd queues (never ScalarE's, which must stay free for exp) and
  prefetch during the k/v projections via staged tile-pool scopes.
"""

import sys

sys.path.insert(0, "/opt/trn_rl_repo")

from collections import deque
from contextlib import ExitStack

import numpy as np
import ml_dtypes

import concourse.tile as tile
from concourse import bacc, mybir

BF16 = mybir.dt.bfloat16
F32 = mybir.dt.float32
P = 128

_PROGRAM_CACHE = {}


def build_mha_program(S=2048, HID=1024, NH=4, DK=64, QB=1024, aug=False):
    """Build + compile the per-core SPMD Bass program."""
    D = NH * DK
    assert NH % 2 == 0 and DK == 64
    SH = S // P
    HT = HID // P
    HTa = HT + (1 if aug else 0)
    QBn = S // QB
    NS = min(512, QB)
    QH = QB // NS
    NQ = S // NS
    DC = D // P
    NPAIR = NH // 2
    GW = DK + 2                  # 64 data cols + rowsum-ones col + pad (4B-aligned groups)

    nc = bacc.Bacc("TRN2", target_bir_lowering=False, debug=False)

    qT_d = nc.dram_tensor("qT", [HTa * P, S], BF16, kind="ExternalInput").ap()
    kT_d = nc.dram_tensor("kT", [HTa * P, S], BF16, kind="ExternalInput").ap()
    vT_d = nc.dram_tensor("vT", [HTa * P, S], BF16, kind="ExternalInput").ap()
    maskT_d = nc.dram_tensor("maskT", [S, S], BF16, kind="ExternalInput").ap()
    wq_d = nc.dram_tensor("wq", [HTa * P, D], BF16, kind="ExternalInput").ap()
    wk_d = nc.dram_tensor("wk", [HTa * P, D], BF16, kind="ExternalInput").ap()
    wv_d = nc.dram_tensor("wv", [HTa * P, D], BF16, kind="ExternalInput").ap()
    wo_d = nc.dram_tensor("wo", [D, HID], BF16, kind="ExternalInput").ap()
    y_d = nc.dram_tensor("y", [HID, S], F32, kind="ExternalOutput").ap()

    Exp = mybir.ActivationFunctionType.Exp

    with tile.TileContext(nc) as tc:
        with ExitStack() as ctx:
            persist = ctx.enter_context(tc.tile_pool(name="persist", bufs=1))
            qh_t = [persist.tile([P, S], BF16, tag=f"qh{d}", name=f"qh{d}")
                    for d in range(DC)]
            kh_t = [persist.tile([P, S], BF16, tag=f"kh{d}", name=f"kh{d}")
                    for d in range(DC)]
            vh_t = [persist.tile([P, NH * GW], BF16, tag=f"vh{s}", name=f"vh{s}")
                    for s in range(SH)]
            xu_t = [persist.tile([P, S], BF16, tag=f"xu{p}", name=f"xu{p}")
                    for p in range(NPAIR)]
            xn_t = [persist.tile([P, S], BF16, tag=f"xn{p}", name=f"xn{p}")
                    for p in range(NPAIR)]
            wo_t = [persist.tile([P, HID], BF16, tag=f"wo{p}", name=f"wo{p}")
                    for p in range(NPAIR)]

            for pr in range(NPAIR):
                nc.sync.dma_start(wo_t[pr][:], wo_d[pr * P:(pr + 1) * P, :])

            def load_inputs(ph, which, src_d, w_d_):
                inp = ph.enter_context(
                    tc.tile_pool(name=f"inp_{which}", bufs=1))
                wp = ph.enter_context(
                    tc.tile_pool(name=f"wp_{which}", bufs=1))
                src_t = [inp.tile([P, S], BF16, tag=f"s{i}",
                                  name=f"{which}T{i}") for i in range(HTa)]
                w_t = [wp.tile([P, D], BF16, tag=f"w{i}",
                               name=f"w_{which}{i}") for i in range(HTa)]
                return src_t, w_t, src_d, w_d_

            def issue_dmas(loaded):
                src_t, w_t, src_d, w_d_ = loaded
                for i in range(HTa):
                    sl = slice(i * P, (i + 1) * P)
                    nc.sync.dma_start(w_t[i][:], w_d_[sl, :])
                    nc.sync.dma_start(src_t[i][:], src_d[sl, :])

            def project(ps1, loaded, dst):
                src_t, w_t, _, _ = loaded
                for dc in range(DC):
                    psl = [ps1.tile([P, NS], F32, tag=f"p1_{qc}",
                                    name=f"p1_{qc}") for qc in range(NQ)]
                    for i in range(HTa):
                        for qc in range(NQ):
                            nc.tensor.matmul(
                                psl[qc][:],
                                w_t[i][:, dc * P:(dc + 1) * P],
                                src_t[i][:, qc * NS:(qc + 1) * NS],
                                start=(i == 0), stop=(i == HTa - 1))
                    for qc in range(NQ):
                        nc.vector.tensor_copy(
                            dst[dc][:, qc * NS:(qc + 1) * NS], psl[qc][:])

            def project_v(ps1, loaded):
                """vh[k, d] with ones cols (rowsum trick) surviving the
                grouped copy."""
                vT_t, wv_t, _, _ = loaded
                for sc in range(SH):
                    ps_v = ps1.tile([P, D], F32, tag="p1v", name="p1v",
                                    bufs=2)
                    for i in range(HTa):
                        nc.tensor.matmul(
                            ps_v[:],
                            vT_t[i][:, sc * P:(sc + 1) * P],
                            wv_t[i][:],
                            start=(i == 0), stop=(i == HTa - 1))
                    nc.vector.memset(vh_t[sc][:], 1.0)
                    dst_v = vh_t[sc][:].rearrange(
                        "p (h c) -> p h c", c=GW)[:, :, 0:DK]
                    src_v = ps_v[:].rearrange("p (h c) -> p h c", c=DK)
                    nc.vector.tensor_copy(dst_v, src_v)

            # phase 1: q,k input DMAs issued up front in priority order on the
            # sync queue (q gates the first matmuls); their pools close before
            # the mask tiles allocate, so SBUF fits (LIFO pool order).
            with ExitStack() as ph_qk:
                lq = load_inputs(ph_qk, "q", qT_d, wq_d)
                lk = load_inputs(ph_qk, "k", kT_d, wk_d)
                issue_dmas(lq)
                issue_dmas(lk)
                ps1 = ph_qk.enter_context(
                    tc.tile_pool(name="ps1", bufs=1, space="PSUM"))
                project(ps1, lq, qh_t)
                project(ps1, lk, kh_t)

            # mask tiles allocate once q/k SBUF is free; their DMAs go on the
            # sync queue strictly AFTER the vT loads so v-proj (which gates
            # the PE) wins the bandwidth race, and ScalarE stays free for exp
            mp = ctx.enter_context(tc.tile_pool(name="mask", bufs=1))
            mask_t = [mp.tile([P, S], BF16, tag=f"m{i}", name=f"m{i}")
                      for i in range(SH)]

            # phase 1b: v projection (masks stream during it)
            with ExitStack() as ph_v:
                lv = load_inputs(ph_v, "v", vT_d, wv_d)
                issue_dmas(lv)
                for i in range(SH):
                    nc.sync.dma_start(mask_t[i][:],
                                      maskT_d[i * P:(i + 1) * P, :])
                ps1v = ph_v.enter_context(
                    tc.tile_pool(name="ps1v", bufs=1, space="PSUM"))
                project_v(ps1v, lv)

            # phase 2+3+4 fused: attention per (q-block, head); after each
            # q-block completes, its normalize + output projection are
            # emitted interleaved into the NEXT q-block's attention as PE
            # filler work (the attention pipeline is ScalarE-bound).
            with ExitStack() as ph2:
                pp = ph2.enter_context(tc.tile_pool(name="pexp", bufs=8))
                pmp = ph2.enter_context(tc.tile_pool(name="pmask", bufs=12))
                rp = ph2.enter_context(tc.tile_pool(name="rp", bufs=2))
                ysb = ph2.enter_context(tc.tile_pool(name="ysb", bufs=8))
                ones_t = persist.tile([P, DK], F32, tag="ones", name="ones")
                nc.vector.memset(ones_t[:], 1.0)
                sps = ph2.enter_context(
                    tc.tile_pool(name="sps", bufs=3, space="PSUM"))
                xps = ph2.enter_context(
                    tc.tile_pool(name="xps", bufs=1, space="PSUM"))

                filler = deque()

                def run_filler(n):
                    for _ in range(n):
                        if not filler:
                            return
                        filler.popleft()()

                def attention_head(qb, h):
                    """QK -> exp -> mask -> AV (deep-lagged) for one head.

                    The x/bc PSUM slot uses a single rotating tag so the score
                    pool can run 3 deep — the PE then almost never blocks on
                    the exp drain, keeping its stream dense (HAM stays at
                    K=8/8).  One filler unit (prev q-block's output
                    projection) runs per chunk."""
                    LAG = 6
                    qsl = slice(qb * QB, (qb + 1) * QB)
                    ht, hb = divmod(h, 2)
                    hsl = slice(64 * hb, 64 * hb + 64)
                    x_ps = xps.tile([P, QB], F32, tag="x", name=f"x{h}")
                    pending = []

                    def emit_av(kc, pm_t):
                        for qh_ in range(QH):
                            nsl = slice(qh_ * NS, (qh_ + 1) * NS)
                            nc.tensor.matmul(
                                x_ps[:DK + 1, nsl],
                                vh_t[kc][:, h * GW:h * GW + DK + 1],
                                pm_t[:, nsl],
                                start=(kc == 0), stop=(kc == SH - 1),
                                skip_group_check=True)

                    for kc in range(SH):
                        s_ps = sps.tile([P, QB], F32, tag="s", name="s")
                        for qh_ in range(QH):
                            nsl = slice(qh_ * NS, (qh_ + 1) * NS)
                            nc.tensor.matmul(
                                s_ps[:, nsl],
                                kh_t[ht][hsl, kc * P:(kc + 1) * P],
                                qh_t[ht][hsl, qb * QB + qh_ * NS:
                                         qb * QB + (qh_ + 1) * NS],
                                start=True, stop=True)
                        p_t = pp.tile([P, QB], BF16, tag="p", name="p")
                        nc.scalar.activation(p_t[:], s_ps[:], Exp, scale=0.125)
                        pm_t = pmp.tile([P, QB], BF16, tag="pm", name="pm")
                        nc.vector.tensor_mul(
                            pm_t[:], p_t[:], mask_t[kc][:, qsl])
                        pending.append((kc, pm_t))
                        # spread the drain over the last chunks so the final
                        # AV flush never starves ScalarE of fresh scores
                        while len(pending) > min(LAG, SH + 1 - kc):
                            emit_av(*pending.pop(0))
                        run_filler(1)
                    for item in pending:
                        emit_av(*item)
                    return x_ps

                def normalize_head(qb, h, x_ps):
                    """r = 1/rowsum off the PSUM row, partition-broadcast by a
                    K=1 fp32 matmul (ones^T @ r) into the recycled x slot,
                    then xn = xu * r.  All on-chip (no DRAM bounce)."""
                    qsl = slice(qb * QB, (qb + 1) * QB)
                    pr, hb = divmod(h, 2)
                    rows = slice(64 * hb, 64 * hb + 64)
                    r_t = rp.tile([DK + 1, QB], F32, tag="r", name=f"r{h}")
                    # single-partition slices break the custom-DVE op
                    # (probe-verified); run it over rows 0..64 — same cost,
                    # only row 64 (the rowsum) is consumed
                    nc.vector.reciprocal_approx_fast(
                        r_t[0:DK + 1, :], x_ps[0:DK + 1, :])
                    nc.vector.tensor_copy(xu_t[pr][rows, qsl], x_ps[:DK, :])
                    bc = sps.tile([P, QB], F32, tag="s", name=f"bc{h}")
                    for qh_ in range(QH):
                        nsl = slice(qh_ * NS, (qh_ + 1) * NS)
                        nc.tensor.matmul(
                            bc[64 * hb:64 * hb + 64, nsl],
                            ones_t[DK:DK + 1, :],
                            r_t[DK:DK + 1, nsl],
                            start=True, stop=True)
                    nc.vector.tensor_mul(
                        xn_t[pr][rows, qsl], xu_t[pr][rows, qsl],
                        bc[rows, :])

                def make_oproj_unit(qb, hc, qh_):
                    def unit():
                        y_ps = sps.tile([P, NS], F32, tag="s", name="yps")
                        qc0 = qb * QH + qh_
                        for pr in range(NPAIR):
                            nc.tensor.matmul(
                                y_ps[:],
                                wo_t[pr][:, hc * P:(hc + 1) * P],
                                xn_t[pr][:, qc0 * NS:(qc0 + 1) * NS],
                                start=(pr == 0), stop=(pr == NPAIR - 1))
                        y_sb = ysb.tile([P, NS], F32, tag="ysb", name="ysb")
                        nc.vector.tensor_copy(y_sb[:], y_ps[:])
                        nc.gpsimd.dma_start(
                            y_d[hc * P:(hc + 1) * P,
                                qc0 * NS:(qc0 + 1) * NS],
                            y_sb[:])
                    return unit

                def make_norm_unit(qb, h, x_ps):
                    def unit():
                        normalize_head(qb, h, x_ps)
                    return unit

                for qb in range(QBn):
                    for h in range(NH):
                        x_ps = attention_head(qb, h)
                        filler.appendleft(make_norm_unit(qb, h, x_ps))
                    for hc in range(HT):
                        for qh_ in range(QH):
                            filler.append(make_oproj_unit(qb, hc, qh_))
                run_filler(len(filler))

    nc.compile()
    return nc


def make_in_maps(q, k, v, mask, Wq, bq, Wk, bk, Wv, bv, Wo,
                 n_cores=8, NH=4, DK=64, aug=False):
    bf = ml_dtypes.bfloat16
    B, S, HID = q.shape
    D = NH * DK
    n_hg = n_cores // B

    def with_aug(xT, bias_row):
        pad = np.zeros((P, xT.shape[1]), xT.dtype)
        pad[0, :] = bias_row
        return np.concatenate([xT, pad], axis=0)

    per_batch = {}
    for b in range(B):
        qT = np.ascontiguousarray(q[b].T).astype(bf)
        kT = np.ascontiguousarray(k[b].T).astype(bf)
        vT = np.ascontiguousarray(v[b].T).astype(bf)
        if aug:
            one = np.ones((S,), np.float32).astype(bf)
            qT, kT, vT = with_aug(qT, one), with_aug(kT, one), with_aug(vT, one)
        per_batch[b] = (qT, kT, vT,
                        np.ascontiguousarray(mask[b, 0].T != 0).astype(bf))

    in_maps = []
    for core in range(n_cores):
        b, hg = divmod(core, n_hg)
        hsl = slice(hg * D, (hg + 1) * D)
        wq = Wq[:, hsl].astype(bf)
        wk = Wk[:, hsl].astype(bf)
        wv = Wv[:, hsl].astype(bf)
        if aug:
            wq = with_aug(wq, bq[hsl].astype(bf))
            wk = with_aug(wk, bk[hsl].astype(bf))
            wv = with_aug(wv, bv[hsl].astype(bf))
        qT, kT, vT, mT = per_batch[b]
        in_maps.append(dict(
            qT=qT, kT=kT, vT=vT, maskT=mT,
            wq=np.ascontiguousarray(wq), wk=np.ascontiguousarray(wk),
            wv=np.ascontiguousarray(wv),
            wo=np.ascontiguousarray(Wo[hsl, :]).astype(bf),
        ))
    return in_maps


def combine_outputs(results, B, S, HID, bo, n_cores=8):
    n_hg = n_cores // B
    out = np.zeros((B, S, HID), np.float32)
    for core in range(n_cores):
        b = core // n_hg
        out[b] += results[core]["y"].T
    return out + bo.astype(np.float32)


def run_mha(q, k, v, mask, Wq, bq, Wk, bk, Wv, bv, Wo, bo, trace=False):
    from concourse.bass_utils import run_bass_kernel_spmd

    B, S, HID = q.shape
    n_cores = 8
    aug = bool(np.any(bq) or np.any(bk) or np.any(bv))
    key = (S, HID, aug)
    if key not in _PROGRAM_CACHE:
        _PROGRAM_CACHE[key] = build_mha_program(S=S, HID=HID, aug=aug)
    nc = _PROGRAM_CACHE[key]
    in_maps = make_in_maps(q, k, v, mask, Wq, bq, Wk, bk, Wv, bv, Wo,
                           n_cores=n_cores, aug=aug)
    res = run_bass_kernel_spmd(nc, in_maps, list(range(n_cores)), trace=trace)
    out = combine_outputs(res.results, B, S, HID, bo, n_cores=n_cores)
    return out, res


def kernel(q, k, v, mask, Wq, bq, Wk, bk, Wv, bv, Wo, bo):
    q = np.asarray(q, np.float32)
    k = np.asarray(k, np.float32)
    v = np.asarray(v, np.float32)
    mask = np.asarray(mask)
    out, _ = run_mha(q, k, v, mask,
                     np.asarray(Wq, np.float32), np.asarray(bq, np.float32),
                     np.asarray(Wk, np.float32), np.asarray(bk, np.float32),
                     np.asarray(Wv, np.float32), np.asarray(bv, np.float32),
                     np.asarray(Wo, np.float32), np.asarray(bo, np.float32))
    return out


# revision 23
# speedup vs baseline: 1.4454x; 1.2628x over previous
"""TRN2 Bass kernel: 16-head MHA (B=2, S=2048, H=1024) sharded over 8 NeuronCores.

Sharding: data-parallel over batch (2) x tensor-parallel over head groups
(4 groups of 4 heads). Each core computes its 4 heads' attention for its batch
and a partial output projection; the host sums the 4 partials per batch,
transposes, and adds the output bias.

Per-core kernel (all activations transposed, bf16 on-chip, fp32 accumulation):
  qhT[d,q] = wq.T @ qT ; khT likewise ; vh[k,d] = (vT.T @ wv) with a ones
  column appended per head (rowsum trick).  Scores are computed transposed
  (s^T[k,q]), exp on ScalarE (scale=1/8 folded in), multiplicative {0,1} mask
  on VectorE, and the AV matmul accumulates x^T[d+1,q] in PSUM where row 64
  is the softmax denominator.  r = 1/rowsum via DVE reciprocal_approx_fast
  straight off the PSUM row, partition-broadcast by an SBUF->SBUF DMA with a
  stride-0 source, then multiplied into x.  Mask tiles are DMAed on the
  sync/gpsimd queues (never ScalarE's, which must stay free for exp) and
  prefetch during the k/v projections via staged tile-pool scopes.
"""

import sys

sys.path.insert(0, "/opt/trn_rl_repo")

from collections import deque
from contextlib import ExitStack

import numpy as np
import ml_dtypes

import concourse.tile as tile
from concourse import bacc, mybir

BF16 = mybir.dt.bfloat16
F32 = mybir.dt.float32
P = 128

_PROGRAM_CACHE = {}


def build_mha_program(S=2048, HID=1024, NH=4, DK=64, QB=1024, aug=False):
    """Build + compile the per-core SPMD Bass program."""
    D = NH * DK
    assert NH % 2 == 0 and DK == 64
    SH = S // P
    HT = HID // P
    HTa = HT + (1 if aug else 0)
    QBn = S // QB
    NS = min(512, QB)
    QH = QB // NS
    NQ = S // NS
    DC = D // P
    NPAIR = NH // 2
    GW = DK + 2                  # 64 data cols + rowsum-ones col + pad (4B-aligned groups)

    nc = bacc.Bacc("TRN2", target_bir_lowering=False, debug=False)

    qT_d = nc.dram_tensor("qT", [HTa * P, S], BF16, kind="ExternalInput").ap()
    kT_d = nc.dram_tensor("kT", [HTa * P, S], BF16, kind="ExternalInput").ap()
    vT_d = nc.dram_tensor("vT", [HTa * P, S], BF16, kind="ExternalInput").ap()
    maskT_d = nc.dram_tensor("maskT", [S, S], BF16, kind="ExternalInput").ap()
    wq_d = nc.dram_tensor("wq", [HTa * P, D], BF16, kind="ExternalInput").ap()
    wk_d = nc.dram_tensor("wk", [HTa * P, D], BF16, kind="ExternalInput").ap()
    wv_d = nc.dram_tensor("wv", [HTa * P, D], BF16, kind="ExternalInput").ap()
    wo_d = nc.dram_tensor("wo", [D, HID], BF16, kind="ExternalInput").ap()
    y_d = nc.dram_tensor("y", [HID, S], F32, kind="ExternalOutput").ap()

    Exp = mybir.ActivationFunctionType.Exp

    with tile.TileContext(nc) as tc:
        with ExitStack() as ctx:
            persist = ctx.enter_context(tc.tile_pool(name="persist", bufs=1))
            qh_t = [persist.tile([P, S], BF16, tag=f"qh{d}", name=f"qh{d}")
                    for d in range(DC)]
            kh_t = [persist.tile([P, S], BF16, tag=f"kh{d}", name=f"kh{d}")
                    for d in range(DC)]
            vh_t = [persist.tile([P, NH * GW], BF16, tag=f"vh{s}", name=f"vh{s}")
                    for s in range(SH)]
            xu_t = [persist.tile([P, S], BF16, tag=f"xu{p}", name=f"xu{p}")
                    for p in range(NPAIR)]
            xn_t = [persist.tile([P, S], BF16, tag=f"xn{p}", name=f"xn{p}")
                    for p in range(NPAIR)]
            wo_t = [persist.tile([P, HID], BF16, tag=f"wo{p}", name=f"wo{p}")
                    for p in range(NPAIR)]

            for pr in range(NPAIR):
                nc.sync.dma_start(wo_t[pr][:], wo_d[pr * P:(pr + 1) * P, :])

            def load_inputs(ph, which, src_d, w_d_):
                inp = ph.enter_context(
                    tc.tile_pool(name=f"inp_{which}", bufs=1))
                wp = ph.enter_context(
                    tc.tile_pool(name=f"wp_{which}", bufs=1))
                src_t = [inp.tile([P, S], BF16, tag=f"s{i}",
                                  name=f"{which}T{i}") for i in range(HTa)]
                w_t = [wp.tile([P, D], BF16, tag=f"w{i}",
                               name=f"w_{which}{i}") for i in range(HTa)]
                return src_t, w_t, src_d, w_d_

            def issue_dmas(loaded):
                src_t, w_t, src_d, w_d_ = loaded
                for i in range(HTa):
                    sl = slice(i * P, (i + 1) * P)
                    nc.sync.dma_start(w_t[i][:], w_d_[sl, :])
                    nc.sync.dma_start(src_t[i][:], src_d[sl, :])

            def project(ps1, loaded, dst):
                src_t, w_t, _, _ = loaded
                for dc in range(DC):
                    psl = [ps1.tile([P, NS], F32, tag=f"p1_{qc}",
                                    name=f"p1_{qc}") for qc in range(NQ)]
                    for i in range(HTa):
                        for qc in range(NQ):
                            nc.tensor.matmul(
                                psl[qc][:],
                                w_t[i][:, dc * P:(dc + 1) * P],
                                src_t[i][:, qc * NS:(qc + 1) * NS],
                                start=(i == 0), stop=(i == HTa - 1))
                    for qc in range(NQ):
                        nc.vector.tensor_copy(
                            dst[dc][:, qc * NS:(qc + 1) * NS], psl[qc][:])

            def project_v(ps1, loaded):
                """vh[k, d] with ones cols (rowsum trick) surviving the
                grouped copy."""
                vT_t, wv_t, _, _ = loaded
                for sc in range(SH):
                    ps_v = ps1.tile([P, D], F32, tag="p1v", name="p1v",
                                    bufs=2)
                    for i in range(HTa):
                        nc.tensor.matmul(
                            ps_v[:],
                            vT_t[i][:, sc * P:(sc + 1) * P],
                            wv_t[i][:],
                            start=(i == 0), stop=(i == HTa - 1))
                    nc.vector.memset(vh_t[sc][:], 1.0)
                    dst_v = vh_t[sc][:].rearrange(
                        "p (h c) -> p h c", c=GW)[:, :, 0:DK]
                    src_v = ps_v[:].rearrange("p (h c) -> p h c", c=DK)
                    nc.vector.tensor_copy(dst_v, src_v)

            # phase 1: q,k input DMAs issued up front in priority order on the
            # sync queue (q gates the first matmuls); their pools close before
            # the mask tiles allocate, so SBUF fits (LIFO pool order).
            with ExitStack() as ph_qk:
                lq = load_inputs(ph_qk, "q", qT_d, wq_d)
                lk = load_inputs(ph_qk, "k", kT_d, wk_d)
                issue_dmas(lq)
                issue_dmas(lk)
                ps1 = ph_qk.enter_context(
                    tc.tile_pool(name="ps1", bufs=1, space="PSUM"))
                project(ps1, lq, qh_t)
                project(ps1, lk, kh_t)

            # mask tiles allocate once q/k SBUF is free; their DMAs go on the
            # sync queue strictly AFTER the vT loads so v-proj (which gates
            # the PE) wins the bandwidth race, and ScalarE stays free for exp
            mp = ctx.enter_context(tc.tile_pool(name="mask", bufs=1))
            mask_t = [mp.tile([P, S], BF16, tag=f"m{i}", name=f"m{i}")
                      for i in range(SH)]

            # phase 1b: v projection (masks stream during it)
            with ExitStack() as ph_v:
                lv = load_inputs(ph_v, "v", vT_d, wv_d)
                issue_dmas(lv)
                for i in range(SH):
                    nc.sync.dma_start(mask_t[i][:],
                                      maskT_d[i * P:(i + 1) * P, :])
                ps1v = ph_v.enter_context(
                    tc.tile_pool(name="ps1v", bufs=1, space="PSUM"))
                project_v(ps1v, lv)

            # phase 2+3+4 fused: attention per (q-block, head); after each
            # q-block completes, its normalize + output projection are
            # emitted interleaved into the NEXT q-block's attention as PE
            # filler work (the attention pipeline is ScalarE-bound).
            with ExitStack() as ph2:
                pp = ph2.enter_context(tc.tile_pool(name="pexp", bufs=8))
                pmp = ph2.enter_context(tc.tile_pool(name="pmask", bufs=12))
                rp = ph2.enter_context(tc.tile_pool(name="rp", bufs=2))
                ysb = ph2.enter_context(tc.tile_pool(name="ysb", bufs=8))
                ones_t = persist.tile([P, DK], F32, tag="ones", name="ones")
                nc.vector.memset(ones_t[:], 1.0)
                sps = ph2.enter_context(
                    tc.tile_pool(name="sps", bufs=3, space="PSUM"))
                xps = ph2.enter_context(
                    tc.tile_pool(name="xps", bufs=1, space="PSUM"))

                filler = deque()

                def run_filler(n):
                    for _ in range(n):
                        if not filler:
                            return
                        filler.popleft()()

                def attention_head(qb, h):
                    """QK -> exp -> mask -> AV (deep-lagged) for one head.

                    The x/bc PSUM slot uses a single rotating tag so the score
                    pool can run 3 deep — the PE then almost never blocks on
                    the exp drain, keeping its stream dense (HAM stays at
                    K=8/8).  One filler unit (prev q-block's output
                    projection) runs per chunk."""
                    LAG = 6
                    qsl = slice(qb * QB, (qb + 1) * QB)
                    ht, hb = divmod(h, 2)
                    hsl = slice(64 * hb, 64 * hb + 64)
                    x_ps = xps.tile([P, QB], F32, tag="x", name=f"x{h}")
                    pending = []

                    def emit_av(kc, pm_t):
                        for qh_ in range(QH):
                            nsl = slice(qh_ * NS, (qh_ + 1) * NS)
                            nc.tensor.matmul(
                                x_ps[:DK + 1, nsl],
                                vh_t[kc][:, h * GW:h * GW + DK + 1],
                                pm_t[:, nsl],
                                start=(kc == 0), stop=(kc == SH - 1),
                                skip_group_check=True)

                    for kc in range(SH):
                        s_ps = sps.tile([P, QB], F32, tag="s", name="s")
                        for qh_ in range(QH):
                            nsl = slice(qh_ * NS, (qh_ + 1) * NS)
                            nc.tensor.matmul(
                                s_ps[:, nsl],
                                kh_t[ht][hsl, kc * P:(kc + 1) * P],
                                qh_t[ht][hsl, qb * QB + qh_ * NS:
                                         qb * QB + (qh_ + 1) * NS],
                                start=True, stop=True)
                        p_t = pp.tile([P, QB], BF16, tag="p", name="p")
                        nc.scalar.activation(p_t[:], s_ps[:], Exp, scale=0.125)
                        pm_t = pmp.tile([P, QB], BF16, tag="pm", name="pm")
                        nc.vector.tensor_mul(
                            pm_t[:], p_t[:], mask_t[kc][:, qsl])
                        pending.append((kc, pm_t))
                        # spread the drain over the last chunks so the final
                        # AV flush never starves ScalarE of fresh scores
                        while len(pending) > min(LAG, SH + 1 - kc):
                            emit_av(*pending.pop(0))
                        run_filler(1)
                    for item in pending:
                        emit_av(*item)
                    return x_ps

                def normalize_head(qb, h, x_ps):
                    """r = 1/rowsum off the PSUM row, partition-broadcast by a
                    K=1 fp32 matmul (ones^T @ r) into the recycled x slot,
                    then xn = xu * r.  All on-chip (no DRAM bounce)."""
                    qsl = slice(qb * QB, (qb + 1) * QB)
                    pr, hb = divmod(h, 2)
                    rows = slice(64 * hb, 64 * hb + 64)
                    r_t = rp.tile([DK + 1, QB], F32, tag="r", name=f"r{h}")
                    # single-partition slices break the custom-DVE op
                    # (probe-verified); run it over rows 0..64 — same cost,
                    # only row 64 (the rowsum) is consumed
                    nc.vector.reciprocal_approx_fast(
                        r_t[0:DK + 1, :], x_ps[0:DK + 1, :])
                    nc.vector.tensor_copy(xu_t[pr][rows, qsl], x_ps[:DK, :])
                    bc = xps.tile([P, QB], F32, tag="x", name=f"bc{h}")
                    for qh_ in range(QH):
                        nsl = slice(qh_ * NS, (qh_ + 1) * NS)
                        nc.tensor.matmul(
                            bc[64 * hb:64 * hb + 64, nsl],
                            ones_t[DK:DK + 1, :],
                            r_t[DK:DK + 1, nsl],
                            start=True, stop=True)
                    nc.vector.tensor_mul(
                        xn_t[pr][rows, qsl], xu_t[pr][rows, qsl],
                        bc[rows, :])

                def make_oproj_unit(qb, hc, qh_, tail=False):
                    def unit():
                        y_ps = sps.tile([P, NS], F32, tag="s", name="yps")
                        qc0 = qb * QH + qh_
                        for pr in range(NPAIR):
                            nc.tensor.matmul(
                                y_ps[:],
                                wo_t[pr][:, hc * P:(hc + 1) * P],
                                xn_t[pr][:, qc0 * NS:(qc0 + 1) * NS],
                                start=(pr == 0), stop=(pr == NPAIR - 1))
                        y_sb = ysb.tile([P, NS], F32, tag="ysb", name="ysb")
                        if tail and hc % 2 == 0:
                            nc.scalar.copy(y_sb[:], y_ps[:])
                        else:
                            nc.vector.tensor_copy(y_sb[:], y_ps[:])
                        nc.gpsimd.dma_start(
                            y_d[hc * P:(hc + 1) * P,
                                qc0 * NS:(qc0 + 1) * NS],
                            y_sb[:])
                    return unit

                for qb in range(QBn):
                    for h in range(NH):
                        x_ps = attention_head(qb, h)
                        normalize_head(qb, h, x_ps)
                    for hc in range(HT):
                        for qh_ in range(QH):
                            filler.append(make_oproj_unit(
                                qb, hc, qh_, tail=(qb == QBn - 1)))
                run_filler(len(filler))

    nc.compile()
    return nc


def make_in_maps(q, k, v, mask, Wq, bq, Wk, bk, Wv, bv, Wo,
                 n_cores=8, NH=4, DK=64, aug=False):
    bf = ml_dtypes.bfloat16
    B, S, HID = q.shape
    D = NH * DK
    n_hg = n_cores // B

    def with_aug(xT, bias_row):
        pad = np.zeros((P, xT.shape[1]), xT.dtype)
        pad[0, :] = bias_row
        return np.concatenate([xT, pad], axis=0)

    per_batch = {}
    for b in range(B):
        qT = np.ascontiguousarray(q[b].T).astype(bf)
        kT = np.ascontiguousarray(k[b].T).astype(bf)
        vT = np.ascontiguousarray(v[b].T).astype(bf)
        if aug:
            one = np.ones((S,), np.float32).astype(bf)
            qT, kT, vT = with_aug(qT, one), with_aug(kT, one), with_aug(vT, one)
        per_batch[b] = (qT, kT, vT,
                        np.ascontiguousarray(mask[b, 0].T != 0).astype(bf))

    in_maps = []
    for core in range(n_cores):
        b, hg = divmod(core, n_hg)
        hsl = slice(hg * D, (hg + 1) * D)
        wq = Wq[:, hsl].astype(bf)
        wk = Wk[:, hsl].astype(bf)
        wv = Wv[:, hsl].astype(bf)
        if aug:
            wq = with_aug(wq, bq[hsl].astype(bf))
            wk = with_aug(wk, bk[hsl].astype(bf))
            wv = with_aug(wv, bv[hsl].astype(bf))
        qT, kT, vT, mT = per_batch[b]
        in_maps.append(dict(
            qT=qT, kT=kT, vT=vT, maskT=mT,
            wq=np.ascontiguousarray(wq), wk=np.ascontiguousarray(wk),
            wv=np.ascontiguousarray(wv),
            wo=np.ascontiguousarray(Wo[hsl, :]).astype(bf),
        ))
    return in_maps


def combine_outputs(results, B, S, HID, bo, n_cores=8):
    n_hg = n_cores // B
    out = np.zeros((B, S, HID), np.float32)
    for core in range(n_cores):
        b = core // n_hg
        out[b] += results[core]["y"].T
    return out + bo.astype(np.float32)


def run_mha(q, k, v, mask, Wq, bq, Wk, bk, Wv, bv, Wo, bo, trace=False):
    from concourse.bass_utils import run_bass_kernel_spmd

    B, S, HID = q.shape
    n_cores = 8
    aug = bool(np.any(bq) or np.any(bk) or np.any(bv))
    key = (S, HID, aug)
    if key not in _PROGRAM_CACHE:
        _PROGRAM_CACHE[key] = build_mha_program(S=S, HID=HID, aug=aug)
    nc = _PROGRAM_CACHE[key]
    in_maps = make_in_maps(q, k, v, mask, Wq, bq, Wk, bk, Wv, bv, Wo,
                           n_cores=n_cores, aug=aug)
    res = run_bass_kernel_spmd(nc, in_maps, list(range(n_cores)), trace=trace)
    out = combine_outputs(res.results, B, S, HID, bo, n_cores=n_cores)
    return out, res


def kernel(q, k, v, mask, Wq, bq, Wk, bk, Wv, bv, Wo, bo):
    q = np.asarray(q, np.float32)
    k = np.asarray(k, np.float32)
    v = np.asarray(v, np.float32)
    mask = np.asarray(mask)
    out, _ = run_mha(q, k, v, mask,
                     np.asarray(Wq, np.float32), np.asarray(bq, np.float32),
                     np.asarray(Wk, np.float32), np.asarray(bk, np.float32),
                     np.asarray(Wv, np.float32), np.asarray(bv, np.float32),
                     np.asarray(Wo, np.float32), np.asarray(bo, np.float32))
    return out
